# revision 7
# baseline (speedup 1.0000x reference)
"""AssocScan Trainium2 kernel: out[:, t] = gates[:, t] * out[:, t-1] + inputs[:, t].

Strategy: the recurrence is independent per (b, d) lane (B*D = 4096 lanes,
N = 4096 steps). The DVE `tensor_tensor_scan` instruction computes exactly
this recurrence along the free dimension, one lane per partition, at a
measured ~2.14 ns/column — that serial rate is the kernel's hard floor
(no other engine supports the scan opcode on NeuronCore v3; gpsimd is
rejected by the ISA engine check at codegen).

Layout: lanes are split across the 8 cores (512 lanes each). On the host,
each core's 512 lanes are packed 4-per-partition, concatenated along the
free dim into one [128, 16384] stream. Because g[:, 0] of every lane
never affects the result (it multiplies the zero initial state), the host
zeroes it; the scan state then self-resets at each lane boundary, so the
whole stream can be scanned with chained tensor_tensor_scan instructions
(the carry passes through the previous segment's last output column).

Pipeline: geometric head segments [256, 512, 1024, 2048, ...] so the
first scan starts as soon as ~65 KB lands (~9.5 us vs ~12.9 us for a
large head), big mid-stream segments to amortize per-instruction
overhead while loads (two HWDGE rings, ~330 B/ns each vs the scan's
~240 B/ns consumption) stay ahead, and a short tail segment so the
final store drain is brief. g rides the sync ring, x the scalar ring;
stores alternate between the two rings (both are idle by the time
stores fire), and the last store is split across both rings to halve
the drain.
"""

import sys

import numpy as np

for _p in ("/opt/trn_rl_repo", "/opt/pypackages"):
    if _p not in sys.path:
        sys.path.append(_p)

import concourse.bacc as bacc
import concourse.mybir as mybir
from concourse.bass_utils import run_bass_kernel_spmd
from concourse.tile import TileContext

B, N, D = 4, 4096, 1024
N_CORES = 8
LANES = B * D                        # 4096 independent (b, d) lanes
LANES_PER_CORE = LANES // N_CORES    # 512
P = 128                              # SBUF partitions
LPP = LANES_PER_CORE // P            # 4 lanes per partition
NC = LPP * N                         # 16384 columns per partition

TRACE = False       # test harness sets True to capture a neuron-profile trace
USE_BF16 = True     # bf16 inputs: quantization ~2e-3 rel, halves load bytes
BF16_OUT = True     # bf16 output stores: halves store bytes
_result_info = {}   # exec_time_ns / trace path from the last run

import os as _os

# Geometric head so the first scan starts early, large middle to amortize
# instruction overhead, short tail to keep the final store off the
# critical path.
# Segment sizing is descriptor-size aware: a [128, s] bf16 transfer has
# 128 descriptors of 2s bytes, and measured per-ring throughput is ~80 B/ns
# at 512 B descriptors, ~130 at 1 KB, ~170 at 2 KB, ~230 at 4 KB, ~350 at
# 8 KB. Tiny head segments are counterproductive — they load slowly AND add
# scan-instruction overhead. 1024-col head (2 KB desc) starts the scan at
# ~11 us; 4096-col bodies stream at full rate.
_SEGS = [int(s) for s in _os.environ.get(
    "SEGS", "1024,2048,4096,4096,4096,1024"
).split(",")]
assert sum(_SEGS) == NC


def _build() -> bacc.Bacc:
    in_dt = mybir.dt.bfloat16 if USE_BF16 else mybir.dt.float32
    out_dt = mybir.dt.bfloat16 if BF16_OUT else mybir.dt.float32
    nc = bacc.Bacc()
    # One contiguous DRAM tensor per segment: every DMA source/dest is a
    # single dense block, which keeps the queues at full descriptor
    # efficiency (column-slicing one big [P, NC] tensor dropped the load
    # rings to ~140 B/ns; dense blocks run ~290-330 B/ns).
    gs = [
        nc.dram_tensor(f"g{k}", [P, seg], in_dt, kind="ExternalInput")
        for k, seg in enumerate(_SEGS)
    ]
    xs = [
        nc.dram_tensor(f"x{k}", [P, seg], in_dt, kind="ExternalInput")
        for k, seg in enumerate(_SEGS)
    ]
    os_ = [
        nc.dram_tensor(f"o{k}", [P, seg], out_dt, kind="ExternalOutput")
        for k, seg in enumerate(_SEGS)
    ]
    M = mybir.AluOpType.mult
    A = mybir.AluOpType.add
    with TileContext(nc) as tc:
        with tc.tile_pool(name="pool", bufs=1) as pool:
            gts = [pool.tile([P, s], in_dt, name=f"gt{k}") for k, s in enumerate(_SEGS)]
            xts = [pool.tile([P, s], in_dt, name=f"xt{k}") for k, s in enumerate(_SEGS)]
            ots = [pool.tile([P, s], out_dt, name=f"ot{k}") for k, s in enumerate(_SEGS)]
            # Load schedule. Empirics from traces: each HWDGE ring sustains
            # ~270-330 B/ns and allows 4 in-flight transfers (sem recycle);
            # two busy rings together reach ~430-450 B/ns of fabric. The
            # scan consumes 256 B per column / 2.14 ns = ~120 B/ns per input
            # stream, so g on the sync ring and x on the scalar ring (one
            # transfer per segment, delivered in scan order) keeps both
            # streams ~2x ahead of the scan. A geometric head gets the first
            # scan going at ~9.7 us; uniform 2048-col bodies keep per-ring
            # transfers big enough to amortize dispatch + sem-recycle stalls.
            for k in range(len(_SEGS)):
                nc.sync.dma_start(out=gts[k][:, :], in_=gs[k][:, :])
                nc.scalar.dma_start(out=xts[k][:, :], in_=xs[k][:, :])
            # Chained scans; the carry crosses segment boundaries through the
            # previous segment's last output column (bf16 rounding there is
            # far inside the error budget). Lane resets happen wherever the
            # host zeroed the gate. Stores alternate between the two rings;
            # they queue behind the loads (ring FIFO) and drain from ~28 us,
            # well before the scan finishes. The final (tiny) store is split
            # across both rings to halve the post-scan drain.
            prev = None
            last = len(_SEGS) - 1
            for k, seg in enumerate(_SEGS):
                init = 0.0 if prev is None else prev
                nc.vector.tensor_tensor_scan(
                    ots[k][:, :], gts[k][:, :], xts[k][:, :], init, M, A
                )
                prev = ots[k][:, seg - 1 : seg]
                if k == last:
                    h = seg // 2
                    nc.sync.dma_start(out=os_[k][:, 0:h], in_=ots[k][:, 0:h])
                    nc.scalar.dma_start(out=os_[k][:, h:seg], in_=ots[k][:, h:seg])
                elif k % 2 == 0:
                    nc.sync.dma_start(out=os_[k][:, :], in_=ots[k][:, :])
                else:
                    nc.scalar.dma_start(out=os_[k][:, :], in_=ots[k][:, :])
    nc.compile()
    return nc


def kernel(gates: np.ndarray, inputs: np.ndarray) -> np.ndarray:
    gates = np.asarray(gates, dtype=np.float32)
    inputs = np.asarray(inputs, dtype=np.float32)

    # Host-side shard: (B, N, D) -> lane-major (B*D, N); row b*D + d is the
    # contiguous time series of lane (b, d). The first gate of every lane
    # multiplies the zero initial state, so it is dead — zero it to make
    # the scan state reset at lane boundaries after concatenation.
    gt = np.ascontiguousarray(gates.transpose(0, 2, 1)).reshape(LANES, N)
    xt = np.ascontiguousarray(inputs.transpose(0, 2, 1)).reshape(LANES, N)
    gt[:, 0] = 0.0
    if USE_BF16:
        import ml_dtypes

        gt = gt.astype(ml_dtypes.bfloat16)
        xt = xt.astype(ml_dtypes.bfloat16)

    # Per core: [512, N] -> [LPP, P, N] -> [P, LPP, N] -> [P, NC]: partition
    # p holds lanes {base + p, base + P + p, ...} concatenated in time.
    # Each column segment ships as its own contiguous array.
    bounds = np.cumsum([0] + _SEGS)
    in_maps = []
    for c in range(N_CORES):
        rows = slice(c * LANES_PER_CORE, (c + 1) * LANES_PER_CORE)
        gc = gt[rows].reshape(LPP, P, N).transpose(1, 0, 2).reshape(P, NC)
        xc = xt[rows].reshape(LPP, P, N).transpose(1, 0, 2).reshape(P, NC)
        m = {}
        for k in range(len(_SEGS)):
            sl = slice(bounds[k], bounds[k + 1])
            m[f"g{k}"] = np.ascontiguousarray(gc[:, sl])
            m[f"x{k}"] = np.ascontiguousarray(xc[:, sl])
        in_maps.append(m)

    nc = _build()
    res = run_bass_kernel_spmd(
        nc, in_maps, core_ids=list(range(N_CORES)), trace=TRACE
    )
    _result_info["exec_time_ns"] = res.exec_time_ns
    _result_info["mean_exec_time_ns"] = res.mean_exec_time_ns
    _result_info["profile_json"] = res.profile_json
    _result_info["trace"] = (
        res.instructions_and_trace[1] if res.instructions_and_trace else None
    )

    # Undo the per-core packing: concat segments -> [P, NC] -> [P, LPP, N]
    # -> [LPP, P, N] -> [512, N], then stack cores back to (LANES, N).
    parts = []
    for c in range(N_CORES):
        oc = np.concatenate(
            [
                res.results[c][f"o{k}"].astype(np.float32, copy=False)
                for k in range(len(_SEGS))
            ],
            axis=1,
        )
        parts.append(
            oc.reshape(P, LPP, N).transpose(1, 0, 2).reshape(LANES_PER_CORE, N)
        )
    out_t = np.concatenate(parts, axis=0)  # (LANES, N)
    return np.ascontiguousarray(out_t.reshape(B, D, N).transpose(0, 2, 1))
